# revision 23
# baseline (speedup 1.0000x reference)
"""Path-signature kernel for Trainium2 (8 NeuronCores, batch-data-parallel).

Computation per batch element b (window W=64, time-augmented dim d=32):
  path  = [linspace(0,1,64) | features[b, t-63:t+1, :]]          (64, 32)
  lvl1  = path[-1] - path[0]                                     (32,)
  inc   = diff(path, axis=0)   prev = path[:-1]                  (63, 32)
  sig2  = inc^T @ prev                                           (32, 32)
  sig3  = einsum('ti,tj,tk->ijk', inc, prev, prev) / 63          (32, 32, 32)
  out   = concat(lvl1, sig2.ravel(), sig3.ravel())               (33824,)

Device mapping (per core, 256 batches):
  - 2 batches are packed per "tile" on the 128 SBUF partitions
    (partition r = b_local*64 + t, with a zero row at t=63 so K=64).
  - lhsT per tile is the block-diagonal (128, 64) increment matrix, so one
    f32r matmul contracts both batches at once (out partitions = (b_local, i)).
  - PP[r, (j,k)] = prev_s[r,j]*prev_s[r,k] (prev_s = prev/sqrt(63)) is built
    by one VectorE tensor_tensor with stride-0 broadcast APs per pair of
    tiles; sig3 = lhsT^T @ PP via two N=512 float32r matmuls per tile.
  - [lvl1 | sig2] comes from one extra N=33 matmul against [ones | prev].
  - Two tiles (4 batches) share each PSUM tensor: tile A -> partitions 0:64,
    tile B -> 64:128 (PE column tiling), so ScalarE PSUM->SBUF copies and the
    final HBM DMAs run at full 128-partition width.
"""

import numpy as np

import concourse.bass as bass
import concourse.mybir as mybir
import concourse.tile as tile
from concourse import bacc
from concourse.bass_utils import run_bass_kernel_spmd

F32 = mybir.dt.float32
F32R = mybir.dt.float32r

N_CORES = 8
B_TOTAL = 2048
T_TOTAL = 1024
F_IN = 31
W = 64
D = 32
B_CORE = B_TOTAL // N_CORES      # 256
N_TILES = B_CORE // 2            # 128  (2 batches per tile)
N_PAIRS = N_TILES // 2           # 64   (4 batches per pair)
OUT_D = D + D * D + D ** 3       # 33824


def build_program(n_pairs=N_PAIRS, mm_dt=mybir.dt.float16, repeat=1):
    """Build the single-core Bass program (SPMD across cores).

    sig3 runs through fp16 matmuls (1 cyc/row on the PE, values are O(10)
    so fp16 range is safe); [lvl1|sig2] uses an exact fp32 matmul (N=33,
    4 cyc/row but tiny).
    """
    n_tiles = 2 * n_pairs
    b_core = 2 * n_tiles
    nc = bacc.Bacc(None, target_bir_lowering=False)

    SQ63 = float(np.sqrt(np.float64(63.0)))

    lhsT16_d = nc.dram_tensor("lhsT16", [128, n_tiles * 64], mm_dt, kind="ExternalInput")
    lhsT32_d = nc.dram_tensor("lhsT32", [128, n_tiles * 64], F32, kind="ExternalInput")
    prevs_d = nc.dram_tensor("prevs", [128, n_tiles * 32], F32, kind="ExternalInput")
    out2_d = nc.dram_tensor("out2", [b_core, D * D], F32, kind="ExternalOutput")
    out3_d = nc.dram_tensor("out3", [b_core, D ** 3], F32, kind="ExternalOutput")

    DMA_SPLIT = 4 if n_tiles % 4 == 0 else 1
    with tile.TileContext(nc) as tc:
        with (
            tc.tile_pool(name="const", bufs=1) as const_pool,
            tc.tile_pool(name="pp", bufs=4) as pp_pool,
            tc.tile_pool(name="s3", bufs=3) as s3_pool,
            tc.tile_pool(name="s2", bufs=1) as s2_pool,
            tc.tile_pool(name="ps3", bufs=3, space=bass.MemorySpace.PSUM) as ps3_pool,
            tc.tile_pool(name="ps2", bufs=2, space=bass.MemorySpace.PSUM) as ps2_pool,
        ):
            lhsT16_all = const_pool.tile([128, n_tiles, 64], mm_dt)
            lhsT32_all = const_pool.tile([128, n_tiles, 64], F32)
            prevs_all = const_pool.tile([128, n_tiles, 32], F32)
            # sig2 staging for the whole core: (128, n_pairs, 32)
            s2_buf = s2_pool.tile([128, n_pairs, 32], F32)

            CHUNK = 8 if n_pairs % 8 == 0 else n_pairs
            n_chunks = n_pairs // CHUNK

            def body():
                q = n_tiles // DMA_SPLIT
                for d in range(DMA_SPLIT):
                    tsl = slice(d * q, (d + 1) * q)
                    nc.sync.dma_start(
                        lhsT16_all[:, tsl, :],
                        lhsT16_d[:, d * q * 64:(d + 1) * q * 64].rearrange("p (t m) -> p t m", m=64))
                    nc.sync.dma_start(
                        lhsT32_all[:, tsl, :],
                        lhsT32_d[:, d * q * 64:(d + 1) * q * 64].rearrange("p (t m) -> p t m", m=64))
                    nc.sync.dma_start(
                        prevs_all[:, tsl, :],
                        prevs_d[:, d * q * 32:(d + 1) * q * 32].rearrange("p (t m) -> p t m", m=32))

                for ch in range(n_chunks):
                    s3_buf = s3_pool.tile([128, CHUNK, 1024], F32, tag="s3buf")
                    for c in range(CHUNK):
                        p = ch * CHUNK + c
                        tA, tB = 2 * p, 2 * p + 1

                        # PP for both tiles in one DVE op: (128, 2, 32, 32),
                        # fp32 inputs, fp16 output (single rounding).
                        pp = pp_pool.tile([128, 2, 32, 32], mm_dt, tag="pp")
                        pv = prevs_all[:, tA:tB + 1, :]          # (128, 2, 32)
                        in0 = pv.unsqueeze(3).broadcast_to([128, 2, 32, 32])
                        in1 = pv.unsqueeze(2).broadcast_to([128, 2, 32, 32])
                        nc.vector.tensor_mul(pp[:], in0, in1)

                        ps3 = ps3_pool.tile([128, 1024], F32, tag="ps3")
                        ps2 = ps2_pool.tile([128, 32], F32, tag="ps2")
                        for half, t in ((0, tA), (1, tB)):
                            lo, hi = 64 * half, 64 * half + 64
                            ppt = pp[:, half]                     # (128, 32, 32)
                            nc.tensor.matmul(
                                ps3[lo:hi, 0:512], lhsT16_all[:, t, :],
                                ppt.rearrange("p j k -> p (j k)")[:, 0:512])
                            nc.tensor.matmul(
                                ps3[lo:hi, 512:1024], lhsT16_all[:, t, :],
                                ppt.rearrange("p j k -> p (j k)")[:, 512:1024])
                            # sig2/sqrt(63) via scaled prev (exact after x sqrt63)
                            nc.tensor.matmul(
                                ps2[lo:hi, 0:32], lhsT32_all[:, t, :],
                                prevs_all[:, t, :])

                        nc.scalar.copy(s3_buf[:, c, :], ps3[:])
                        nc.scalar.activation(
                            s2_buf[:, ch * CHUNK + c, :], ps2[:],
                            mybir.ActivationFunctionType.Copy, scale=SQ63)

                    # sig3 out: b = (ch*CHUNK + c)*4 + s ; partition = (s, i)
                    v3 = out3_d[:].rearrange(
                        "(ch c s) (i m) -> ch s i c m",
                        ch=n_chunks, c=CHUNK, s=4, i=32, m=1024)
                    nc.sync.dma_start(v3[ch], s3_buf[:])

                # sig2 out, once at the end
                v2 = out2_d[:].rearrange(
                    "(c s) (i k) -> s i c k", c=n_pairs, s=4, i=32, k=32)
                nc.sync.dma_start(v2, s2_buf[:])

            for _rep in range(repeat):
                body()

    nc.compile()
    return nc


def make_inputs_for_core(inc, prev, prev_s, base, n_tiles):
    """Pack host arrays into the partition-major device layouts.

    inc/prev: (B, 64, 32) with zero row at t=63; prev_s = prev/sqrt(63).
    """
    nt = n_tiles
    lhsT = np.zeros((128, nt, 64), dtype=np.float32)
    prevs = np.zeros((128, nt, 32), dtype=np.float32)

    sl = slice(base, base + 2 * nt)
    # (nt, 2, 64, 32) -> per bl: (64, nt, 32)
    A = inc[sl].reshape(nt, 2, 64, 32).transpose(1, 2, 0, 3)
    S = prev_s[sl].reshape(nt, 2, 64, 32).transpose(1, 2, 0, 3)
    for bl in range(2):
        rows = slice(64 * bl, 64 * bl + 64)
        lhsT[rows, :, 32 * bl:32 * bl + 32] = A[bl]
        prevs[rows, :, :] = S[bl]
    return {
        "lhsT16": lhsT.reshape(128, nt * 64).astype(np.float16),
        "lhsT32": lhsT.reshape(128, nt * 64),
        "prevs": prevs.reshape(128, nt * 32),
    }


def host_preprocess(features, t):
    t = int(t)
    start = max(0, t - W + 1)
    window = features[:, start:t + 1, :]
    cur = window.shape[1]
    if cur < W:
        pad = np.broadcast_to(window[:, 0:1, :], (window.shape[0], W - cur, F_IN))
        window = np.concatenate([pad, window], axis=1)
    B = window.shape[0]
    path = np.empty((B, W, D), dtype=np.float32)
    path[:, :, 0] = np.linspace(0.0, 1.0, W, dtype=np.float32)[None, :]
    path[:, :, 1:] = window

    prev = np.zeros((B, W, D), dtype=np.float32)
    prev[:, :W - 1] = path[:, :W - 1]
    inc = np.zeros((B, W, D), dtype=np.float32)
    inc[:, :W - 1] = path[:, 1:] - path[:, :-1]
    prev_s = prev * np.float32(1.0 / np.sqrt(np.float32(W - 1)))
    lvl1 = path[:, -1, :] - path[:, 0, :]
    return inc, prev, prev_s, lvl1


_PROGRAM = None


def run(features, t, trace=False):
    global _PROGRAM
    features = np.asarray(features, dtype=np.float32)
    inc, prev, prev_s, lvl1 = host_preprocess(features, t)

    if _PROGRAM is None:
        _PROGRAM = build_program()
    nc = _PROGRAM

    in_maps = [
        make_inputs_for_core(inc, prev, prev_s, c * B_CORE, N_TILES)
        for c in range(N_CORES)
    ]
    res = run_bass_kernel_spmd(nc, in_maps, list(range(N_CORES)), trace=trace)
    out = np.empty((B_TOTAL, OUT_D), dtype=np.float32)
    out[:, 0:D] = lvl1
    for c in range(N_CORES):
        rows = slice(c * B_CORE, (c + 1) * B_CORE)
        out[rows, D:D + D * D] = res.results[c]["out2"]
        out[rows, D + D * D:] = res.results[c]["out3"]
    return out, res


def kernel(features, t):
    return run(features, t)[0]


# revision 25
# speedup vs baseline: 2265.7599x; 2265.7599x over previous
"""Path-signature kernel for Trainium2 (8 NeuronCores, batch-data-parallel).

Computation per batch element b (window W=64, time-augmented dim d=32):
  path  = [linspace(0,1,64) | features[b, t-63:t+1, :]]          (64, 32)
  lvl1  = path[-1] - path[0]                                     (32,)
  inc   = diff(path, axis=0)   prev = path[:-1]                  (63, 32)
  sig2  = inc^T @ prev                                           (32, 32)
  sig3  = einsum('ti,tj,tk->ijk', inc, prev, prev) / 63          (32, 32, 32)
  out   = concat(lvl1, sig2.ravel(), sig3.ravel())               (33824,)

Device mapping (per core, 256 batches):
  - 2 batches are packed per "tile" on the 128 SBUF partitions
    (partition r = b_local*64 + t, with a zero row at t=63 so K=64).
  - lhsT per tile is the block-diagonal (128, 64) increment matrix, so one
    f32r matmul contracts both batches at once (out partitions = (b_local, i)).
  - PP[r, (j,k)] = prev_s[r,j]*prev_s[r,k] (prev_s = prev/sqrt(63)) is built
    by one VectorE tensor_tensor with stride-0 broadcast APs per pair of
    tiles; sig3 = lhsT^T @ PP via two N=512 float32r matmuls per tile.
  - [lvl1 | sig2] comes from one extra N=33 matmul against [ones | prev].
  - Two tiles (4 batches) share each PSUM tensor: tile A -> partitions 0:64,
    tile B -> 64:128 (PE column tiling), so ScalarE PSUM->SBUF copies and the
    final HBM DMAs run at full 128-partition width.
"""

import numpy as np

import concourse.bass as bass
import concourse.mybir as mybir
import concourse.tile as tile
from concourse import bacc
from concourse.bass_utils import run_bass_kernel_spmd

F32 = mybir.dt.float32
F32R = mybir.dt.float32r

N_CORES = 8
B_TOTAL = 2048
T_TOTAL = 1024
F_IN = 31
W = 64
D = 32
B_CORE = B_TOTAL // N_CORES      # 256
N_TILES = B_CORE // 2            # 128  (2 batches per tile)
N_PAIRS = N_TILES // 2           # 64   (4 batches per pair)
OUT_D = D + D * D + D ** 3       # 33824


def build_program(n_pairs=N_PAIRS, mm_dt=mybir.dt.float16, repeat=1, loop=0):
    """Build the single-core Bass program (SPMD across cores).

    sig3 runs through fp16 matmuls (1 cyc/row on the PE, values are O(10)
    so fp16 range is safe); [lvl1|sig2] uses an exact fp32 matmul (N=33,
    4 cyc/row but tiny).
    """
    n_tiles = 2 * n_pairs
    b_core = 2 * n_tiles
    nc = bacc.Bacc(None, target_bir_lowering=False)

    SQ63 = float(np.sqrt(np.float64(63.0)))

    lhsT16_d = nc.dram_tensor("lhsT16", [128, n_tiles * 64], mm_dt, kind="ExternalInput")
    lhsT32_d = nc.dram_tensor("lhsT32", [128, n_tiles * 64], F32, kind="ExternalInput")
    prevs_d = nc.dram_tensor("prevs", [128, n_tiles * 32], F32, kind="ExternalInput")
    out2_d = nc.dram_tensor("out2", [b_core, D * D], F32, kind="ExternalOutput")
    out3_d = nc.dram_tensor("out3", [b_core, D ** 3], F32, kind="ExternalOutput")

    DMA_SPLIT = 4 if n_tiles % 4 == 0 else 1
    with tile.TileContext(nc) as tc:
        with (
            tc.tile_pool(name="const", bufs=1) as const_pool,
            tc.tile_pool(name="pp", bufs=4) as pp_pool,
            tc.tile_pool(name="s3", bufs=3) as s3_pool,
            tc.tile_pool(name="s2", bufs=1) as s2_pool,
            tc.tile_pool(name="ps3", bufs=3, space=bass.MemorySpace.PSUM) as ps3_pool,
            tc.tile_pool(name="ps2", bufs=2, space=bass.MemorySpace.PSUM) as ps2_pool,
        ):
            lhsT16_all = const_pool.tile([128, n_tiles, 64], mm_dt)
            lhsT32_all = const_pool.tile([128, n_tiles, 64], F32)
            prevs_all = const_pool.tile([128, n_tiles, 32], F32)
            # sig2 staging for the whole core: (128, n_pairs, 32)
            s2_buf = s2_pool.tile([128, n_pairs, 32], F32)

            CHUNK = 8 if n_pairs % 8 == 0 else n_pairs
            n_chunks = n_pairs // CHUNK

            def body():
                q = n_tiles // DMA_SPLIT
                for d in range(DMA_SPLIT):
                    tsl = slice(d * q, (d + 1) * q)
                    nc.sync.dma_start(
                        lhsT16_all[:, tsl, :],
                        lhsT16_d[:, d * q * 64:(d + 1) * q * 64].rearrange("p (t m) -> p t m", m=64))
                    nc.sync.dma_start(
                        lhsT32_all[:, tsl, :],
                        lhsT32_d[:, d * q * 64:(d + 1) * q * 64].rearrange("p (t m) -> p t m", m=64))
                    nc.sync.dma_start(
                        prevs_all[:, tsl, :],
                        prevs_d[:, d * q * 32:(d + 1) * q * 32].rearrange("p (t m) -> p t m", m=32))

                for ch in range(n_chunks):
                    s3_buf = s3_pool.tile([128, CHUNK, 1024], F32, tag="s3buf")
                    for c in range(CHUNK):
                        p = ch * CHUNK + c
                        tA, tB = 2 * p, 2 * p + 1

                        # PP for both tiles in one DVE op: (128, 2, 32, 32),
                        # fp32 inputs, fp16 output (single rounding).
                        pp = pp_pool.tile([128, 2, 32, 32], mm_dt, tag="pp")
                        pv = prevs_all[:, tA:tB + 1, :]          # (128, 2, 32)
                        in0 = pv.unsqueeze(3).broadcast_to([128, 2, 32, 32])
                        in1 = pv.unsqueeze(2).broadcast_to([128, 2, 32, 32])
                        nc.vector.tensor_mul(pp[:], in0, in1)

                        ps3 = ps3_pool.tile([128, 1024], F32, tag="ps3")
                        ps2 = ps2_pool.tile([128, 32], F32, tag="ps2")
                        for half, t in ((0, tA), (1, tB)):
                            lo, hi = 64 * half, 64 * half + 64
                            ppt = pp[:, half]                     # (128, 32, 32)
                            nc.tensor.matmul(
                                ps3[lo:hi, 0:512], lhsT16_all[:, t, :],
                                ppt.rearrange("p j k -> p (j k)")[:, 0:512])
                            nc.tensor.matmul(
                                ps3[lo:hi, 512:1024], lhsT16_all[:, t, :],
                                ppt.rearrange("p j k -> p (j k)")[:, 512:1024])
                            # sig2/sqrt(63) via scaled prev (exact after x sqrt63)
                            nc.tensor.matmul(
                                ps2[lo:hi, 0:32], lhsT32_all[:, t, :],
                                prevs_all[:, t, :])

                        nc.scalar.copy(s3_buf[:, c, :], ps3[:])
                        nc.scalar.activation(
                            s2_buf[:, ch * CHUNK + c, :], ps2[:],
                            mybir.ActivationFunctionType.Copy, scale=SQ63)

                    # sig3 out: b = (ch*CHUNK + c)*4 + s ; partition = (s, i)
                    v3 = out3_d[:].rearrange(
                        "(ch c s) (i m) -> ch s i c m",
                        ch=n_chunks, c=CHUNK, s=4, i=32, m=1024)
                    nc.sync.dma_start(v3[ch], s3_buf[:])

                # sig2 out, once at the end
                v2 = out2_d[:].rearrange(
                    "(c s) (i k) -> s i c k", c=n_pairs, s=4, i=32, k=32)
                nc.sync.dma_start(v2, s2_buf[:])

            if loop:
                with tc.For_i(0, loop, 1):
                    body()
            else:
                for _rep in range(repeat):
                    body()

    nc.compile()
    return nc


def make_inputs_for_core(inc, prev, prev_s, base, n_tiles):
    """Pack host arrays into the partition-major device layouts.

    inc/prev: (B, 64, 32) with zero row at t=63; prev_s = prev/sqrt(63).
    """
    nt = n_tiles
    lhsT = np.zeros((128, nt, 64), dtype=np.float32)
    prevs = np.zeros((128, nt, 32), dtype=np.float32)

    sl = slice(base, base + 2 * nt)
    # (nt, 2, 64, 32) -> per bl: (64, nt, 32)
    A = inc[sl].reshape(nt, 2, 64, 32).transpose(1, 2, 0, 3)
    S = prev_s[sl].reshape(nt, 2, 64, 32).transpose(1, 2, 0, 3)
    for bl in range(2):
        rows = slice(64 * bl, 64 * bl + 64)
        lhsT[rows, :, 32 * bl:32 * bl + 32] = A[bl]
        prevs[rows, :, :] = S[bl]
    return {
        "lhsT16": lhsT.reshape(128, nt * 64).astype(np.float16),
        "lhsT32": lhsT.reshape(128, nt * 64),
        "prevs": prevs.reshape(128, nt * 32),
    }


def host_preprocess(features, t):
    t = int(t)
    start = max(0, t - W + 1)
    window = features[:, start:t + 1, :]
    cur = window.shape[1]
    if cur < W:
        pad = np.broadcast_to(window[:, 0:1, :], (window.shape[0], W - cur, F_IN))
        window = np.concatenate([pad, window], axis=1)
    B = window.shape[0]
    path = np.empty((B, W, D), dtype=np.float32)
    path[:, :, 0] = np.linspace(0.0, 1.0, W, dtype=np.float32)[None, :]
    path[:, :, 1:] = window

    prev = np.zeros((B, W, D), dtype=np.float32)
    prev[:, :W - 1] = path[:, :W - 1]
    inc = np.zeros((B, W, D), dtype=np.float32)
    inc[:, :W - 1] = path[:, 1:] - path[:, :-1]
    prev_s = prev * np.float32(1.0 / np.sqrt(np.float32(W - 1)))
    lvl1 = path[:, -1, :] - path[:, 0, :]
    return inc, prev, prev_s, lvl1


_PROGRAM = None


def run(features, t, trace=False):
    global _PROGRAM
    features = np.asarray(features, dtype=np.float32)
    inc, prev, prev_s, lvl1 = host_preprocess(features, t)

    if _PROGRAM is None:
        _PROGRAM = build_program()
    nc = _PROGRAM

    in_maps = [
        make_inputs_for_core(inc, prev, prev_s, c * B_CORE, N_TILES)
        for c in range(N_CORES)
    ]
    res = run_bass_kernel_spmd(nc, in_maps, list(range(N_CORES)), trace=trace)
    out = np.empty((B_TOTAL, OUT_D), dtype=np.float32)
    out[:, 0:D] = lvl1
    for c in range(N_CORES):
        rows = slice(c * B_CORE, (c + 1) * B_CORE)
        out[rows, D:D + D * D] = res.results[c]["out2"]
        out[rows, D + D * D:] = res.results[c]["out3"]
    return out, res


def kernel(features, t):
    return run(features, t)[0]


# revision 36
# speedup vs baseline: 2862.3057x; 1.2633x over previous
"""Path-signature kernel for Trainium2 (8 NeuronCores, batch-data-parallel).

Computation per batch element b (window W=64, time-augmented dim d=32):
  path  = [linspace(0,1,64) | features[b, t-63:t+1, :]]          (64, 32)
  lvl1  = path[-1] - path[0]                                     (32,)
  inc   = diff(path, axis=0)   prev = path[:-1]                  (63, 32)
  sig2  = inc^T @ prev                                           (32, 32)
  sig3  = einsum('ti,tj,tk->ijk', inc, prev, prev) / 63          (32, 32, 32)
  out   = concat(lvl1, sig2.ravel(), sig3.ravel())               (33824,)

Device mapping (per core, 256 batches):
  - 2 batches packed per "tile" on the 128 SBUF partitions
    (partition r = b_local*64 + t, zero row at t=63 so K=64).
  - lhsT per tile is the block-diagonal (128, 64) fp16 increment matrix, so
    one matmul contracts both batches (out partitions = (b_local, i)).
  - prevx holds [prev/sqrt(63) | const 1/sqrt(63)] (33 channels).  One
    VectorE tensor_tensor with stride-0 broadcast APs builds
    PP[r,(j,k')] = prev_s[r,j] * prevx[r,k']  (32x33, fp16 out); then
    lhsT^T @ PP gives sig3 in columns k'<32 and sig2/63 in column k'=32 —
    sig2 needs no separate input or fp32 matmul.
  - Two tiles (4 batches) share each PSUM tensor: tile A -> partitions 0:64,
    tile B -> 64:128 (PE column tiling), so ScalarE PSUM->SBUF copies and
    the final HBM DMAs run at full 128-partition width.
  - lvl1 is a single host-side subtraction (0.1% of the output).
"""

import numpy as np

import concourse.bass as bass
import concourse.mybir as mybir
import concourse.tile as tile
from concourse import bacc
from concourse.bass_utils import run_bass_kernel_spmd

F32 = mybir.dt.float32
F16 = mybir.dt.float16

N_CORES = 8
B_TOTAL = 2048
T_TOTAL = 1024
F_IN = 31
W = 64
D = 32
B_CORE = B_TOTAL // N_CORES      # 256
N_TILES = B_CORE // 2            # 128  (2 batches per tile)
N_PAIRS = N_TILES // 2           # 64   (4 batches per pair)
OUT_D = D + D * D + D ** 3       # 33824


def build_program(n_pairs=N_PAIRS, mm_dt=mybir.dt.float16, repeat=1, loop=0,
                  variant="full", chunk=4, tri=False):
    """Build the single-core Bass program (SPMD across cores)."""
    n_tiles = 2 * n_pairs
    b_core = 2 * n_tiles
    nc = bacc.Bacc(None, target_bir_lowering=False)

    lhsT16_d = nc.dram_tensor("lhsT16", [128, n_tiles * 64], mm_dt, kind="ExternalInput")
    prevx_d = nc.dram_tensor("prevx", [128, n_tiles * 33], F32, kind="ExternalInput")
    out2_d = nc.dram_tensor("out2", [b_core, D * D], F32, kind="ExternalOutput")
    out3_d = nc.dram_tensor("out3", [b_core, D ** 3], F32, kind="ExternalOutput")

    DMA_SPLIT = 8 if n_tiles % 8 == 0 else 1
    with tile.TileContext(nc) as tc:
        with (
            tc.tile_pool(name="const", bufs=1) as const_pool,
            tc.tile_pool(name="pp", bufs=4) as pp_pool,
            tc.tile_pool(name="s3", bufs=3) as s3_pool,
            tc.tile_pool(name="s2", bufs=1) as s2_pool,
            tc.tile_pool(name="ps3", bufs=2, space=bass.MemorySpace.PSUM) as ps3_pool,
        ):
            lhsT16_all = const_pool.tile([128, n_tiles, 64], mm_dt)
            prevx_all = const_pool.tile([128, n_tiles, 33], F32)
            # sig2 staging for the whole core: (128, n_pairs, 32)
            s2_buf = s2_pool.tile([128, n_pairs, 32], F32)

            CHUNK = chunk if n_pairs % chunk == 0 else n_pairs
            n_chunks = n_pairs // CHUNK

            def body():
                q = n_tiles // DMA_SPLIT
                for d in range(DMA_SPLIT):
                    tsl = slice(d * q, (d + 1) * q)
                    nc.sync.dma_start(
                        prevx_all[:, tsl, :],
                        prevx_d[:, d * q * 33:(d + 1) * q * 33].rearrange("p (t m) -> p t m", m=33))
                    nc.sync.dma_start(
                        lhsT16_all[:, tsl, :],
                        lhsT16_d[:, d * q * 64:(d + 1) * q * 64].rearrange("p (t m) -> p t m", m=64))

                for ch in range(n_chunks):
                    s3_buf = s3_pool.tile([128, CHUNK, 1024], F32, tag="s3buf")
                    if tri:
                        # One pp tile per chunk, only the (j<=k)-ish blocks:
                        # P0 j<16,k'<16 | P1 j<16,k'16:33 | P2 j16:32,k'16:33
                        nt2 = 2 * CHUNK
                        t0 = 2 * ch * CHUNK
                        pp = pp_pool.tile([128, nt2, 800], mm_dt, tag="pp")
                        px = prevx_all[:, t0:t0 + nt2, :]
                        for (js, ks, lo_c, wk) in (
                                ((0, 16), (0, 16), 0, 16),
                                ((0, 16), (16, 33), 256, 17),
                                ((16, 32), (16, 33), 528, 17)):
                            wj = js[1] - js[0]
                            in0 = px[:, :, js[0]:js[1]].unsqueeze(3).broadcast_to(
                                [128, nt2, wj, wk])
                            in1 = px[:, :, ks[0]:ks[1]].unsqueeze(2).broadcast_to(
                                [128, nt2, wj, wk])
                            out = pp[:, :, lo_c:lo_c + wj * wk].rearrange(
                                "p t (j k) -> p t j k", k=wk)
                            nc.vector.tensor_mul(out, in0, in1)
                    for c in range(CHUNK):
                        p = ch * CHUNK + c
                        tA, tB = 2 * p, 2 * p + 1

                        if not tri:
                            # PP for both tiles in one DVE op: (128,2,32,33),
                            # fp32 inputs, fp16 output (single rounding).
                            pp = pp_pool.tile([128, 2, 32, 33], mm_dt, tag="pp")
                            pj = prevx_all[:, tA:tB + 1, 0:32]   # (128, 2, 32)
                            pk = prevx_all[:, tA:tB + 1, 0:33]   # (128, 2, 33)
                            in0 = pj.unsqueeze(3).broadcast_to([128, 2, 32, 33])
                            in1 = pk.unsqueeze(2).broadcast_to([128, 2, 32, 33])
                            nc.vector.tensor_mul(pp[:], in0, in1)

                        if tri:
                            psA = ps3_pool.tile([128, 256], F32, tag="psA")
                            psB = ps3_pool.tile([128, 272], F32, tag="psB")
                            psC = ps3_pool.tile([128, 272], F32, tag="psC")
                            for half, t in ((0, tA), (1, tB)):
                                lo, hi = 64 * half, 64 * half + 64
                                tloc = 2 * c + half
                                w = lhsT16_all[:, t, :]
                                nc.tensor.matmul(psA[lo:hi, :], w, pp[:, tloc, 0:256])
                                nc.tensor.matmul(psB[lo:hi, :], w, pp[:, tloc, 256:528])
                                nc.tensor.matmul(psC[lo:hi, :], w, pp[:, tloc, 528:800])
                            s3v = s3_buf[:, c, :].rearrange("p (j k) -> p j k", k=32)
                            psBv = psB[:].rearrange("p (j k) -> p j k", k=17)
                            psCv = psC[:].rearrange("p (j k) -> p j k", k=17)
                            # (j<16, k<16)
                            nc.scalar.copy(
                                s3v[:, 0:16, 0:16],
                                psA[:].rearrange("p (j k) -> p j k", k=16))
                            # (j<16, k 16:32)
                            nc.scalar.copy(s3v[:, 0:16, 16:32], psBv[:, :, 0:16])
                            # (j 16:32, k 16:32)
                            nc.scalar.copy(s3v[:, 16:32, 16:32], psCv[:, :, 0:16])
                            # mirror: (j 16:32, k<16) = psB[(k, j)]
                            nc.scalar.copy(
                                s3v[:, 16:32, 0:16],
                                psBv[:, :, 0:16].transpose([0, 2, 1]))
                            # sig2/63 columns
                            nc.scalar.activation(
                                s2_buf[:, p, 0:16], psBv[:, :, 16],
                                mybir.ActivationFunctionType.Copy, scale=63.0)
                            nc.scalar.activation(
                                s2_buf[:, p, 16:32], psCv[:, :, 16],
                                mybir.ActivationFunctionType.Copy, scale=63.0)
                        else:
                            ps3 = ps3_pool.tile([128, 1056], F32, tag="ps3")
                            for half, t in ((0, tA), (1, tB)):
                                lo, hi = 64 * half, 64 * half + 64
                                ppf = pp[:, half].rearrange("p j k -> p (j k)")
                                nc.tensor.matmul(
                                    ps3[lo:hi, 0:512], lhsT16_all[:, t, :],
                                    ppf[:, 0:512])
                                nc.tensor.matmul(
                                    ps3[lo:hi, 512:1024], lhsT16_all[:, t, :],
                                    ppf[:, 512:1024])
                                nc.tensor.matmul(
                                    ps3[lo:hi, 1024:1056], lhsT16_all[:, t, :],
                                    ppf[:, 1024:1056])

                            # sig3: cols j*33+k, k<32 ; sig2/63: cols j*33+32
                            ps3v = ps3[:].rearrange("p (j k) -> p j k", k=33)
                            nc.scalar.copy(
                                s3_buf[:, c, :].rearrange("p (j k) -> p j k", k=32),
                                ps3v[:, :, 0:32])
                            nc.scalar.activation(
                                s2_buf[:, p, :], ps3v[:, :, 32],
                                mybir.ActivationFunctionType.Copy, scale=63.0)

                    # sig3 out: b = (ch*CHUNK + c)*4 + s ; partition = (s, i)
                    if variant != "nodma3":
                        v3 = out3_d[:].rearrange(
                            "(ch c s) (i m) -> ch s i c m",
                            ch=n_chunks, c=CHUNK, s=4, i=32, m=1024)
                        nc.sync.dma_start(v3[ch], s3_buf[:])

                # sig2 out, once at the end
                v2 = out2_d[:].rearrange(
                    "(c s) (i k) -> s i c k", c=n_pairs, s=4, i=32, k=32)
                nc.sync.dma_start(v2, s2_buf[:])

            if loop:
                with tc.For_i(0, loop, 1):
                    body()
            else:
                for _rep in range(repeat):
                    body()

    nc.compile()
    return nc


def make_inputs_for_core(inc, prev_s, base, n_tiles):
    """Pack host arrays into the partition-major device layouts.

    inc: (B, 64, 32) with zero row at t=63; prev_s = prev/sqrt(63) likewise.
    """
    nt = n_tiles
    lhsT = np.zeros((128, nt, 64), dtype=np.float32)
    prevx = np.zeros((128, nt, 33), dtype=np.float32)

    sl = slice(base, base + 2 * nt)
    # (nt, 2, 64, 32) -> per bl: (64, nt, 32)
    A = inc[sl].reshape(nt, 2, 64, 32).transpose(1, 2, 0, 3)
    S = prev_s[sl].reshape(nt, 2, 64, 32).transpose(1, 2, 0, 3)
    c0 = np.float32(1.0 / np.sqrt(np.float64(63.0)))
    for bl in range(2):
        rows = slice(64 * bl, 64 * bl + 64)
        lhsT[rows, :, 32 * bl:32 * bl + 32] = A[bl]
        prevx[rows, :, 0:32] = S[bl]
        prevx[64 * bl:64 * bl + 63, :, 32] = c0  # zero at the pad row
    return {
        "lhsT16": lhsT.reshape(128, nt * 64).astype(np.float16),
        "prevx": prevx.reshape(128, nt * 33),
    }


def host_preprocess(features, t):
    t = int(t)
    start = max(0, t - W + 1)
    window = features[:, start:t + 1, :]
    cur = window.shape[1]
    if cur < W:
        pad = np.broadcast_to(window[:, 0:1, :], (window.shape[0], W - cur, F_IN))
        window = np.concatenate([pad, window], axis=1)
    B = window.shape[0]
    path = np.empty((B, W, D), dtype=np.float32)
    path[:, :, 0] = np.linspace(0.0, 1.0, W, dtype=np.float32)[None, :]
    path[:, :, 1:] = window

    inc = np.zeros((B, W, D), dtype=np.float32)
    inc[:, :W - 1] = path[:, 1:] - path[:, :-1]
    prev_s = np.zeros((B, W, D), dtype=np.float32)
    prev_s[:, :W - 1] = path[:, :W - 1] * np.float32(1.0 / np.sqrt(np.float32(W - 1)))
    lvl1 = path[:, -1, :] - path[:, 0, :]
    return inc, prev_s, lvl1


_PROGRAM = None


def run(features, t, trace=False):
    global _PROGRAM
    features = np.asarray(features, dtype=np.float32)
    inc, prev_s, lvl1 = host_preprocess(features, t)

    if _PROGRAM is None:
        _PROGRAM = build_program()
    nc = _PROGRAM

    in_maps = [
        make_inputs_for_core(inc, prev_s, c * B_CORE, N_TILES)
        for c in range(N_CORES)
    ]
    res = run_bass_kernel_spmd(nc, in_maps, list(range(N_CORES)), trace=trace)
    out = np.empty((B_TOTAL, OUT_D), dtype=np.float32)
    out[:, 0:D] = lvl1
    for c in range(N_CORES):
        rows = slice(c * B_CORE, (c + 1) * B_CORE)
        out[rows, D:D + D * D] = res.results[c]["out2"]
        out[rows, D + D * D:] = res.results[c]["out3"]
    return out, res


def kernel(features, t):
    return run(features, t)[0]
